# revision 5
# baseline (speedup 1.0000x reference)
"""Trainium2 Bass kernel v2 for nn_Block_40879498729310 (GPT-style block).

Sharding: TP=4 over heads x DP=2 over batches (8 cores), as v1, but:
- bf16 everywhere off the accumulate path (weights, activations, collectives);
  PSUM accumulation and residual stream stay f32. Verified rel err ~3e-3.
- qc-major attention: for each 512-row block, all 3 heads' attention, then the
  proj partial and its (bf16) ReduceScatter are issued immediately, so the 4
  collectives (15us fixed cost each) overlap the remaining attention compute.
- Each core's MLP rows are the strided set {qc*512 + r*128 + j}; LN2/transpose
  for a row tile starts as soon as its ReduceScatter lands; fc1 runs in two
  256-row halves (cols ready after RS1/RS3); fc2 streams its weights.
- Causal masking is multiplicative post-exp on the single mixed 128x128
  diagonal block per k-chunk; fully-invalid score columns are never computed
  (triangular trimming).
"""
import numpy as np
from contextlib import ExitStack
from functools import lru_cache

import ml_dtypes
import concourse.bass as bass
import concourse.mybir as mybir
import concourse.tile as tile
from concourse.bass_utils import run_bass_kernel_spmd
from concourse.masks import make_identity

F32 = mybir.dt.float32
BF16 = mybir.dt.bfloat16
AF = mybir.ActivationFunctionType
OP = mybir.AluOpType
NPBF16 = ml_dtypes.bfloat16

N_EMBD = 768
N_HEAD = 12
B = 2
T = 2048
HD = 64
GROUP = 4                 # TP group size
HPC = N_HEAD // GROUP     # heads per core = 3
ROWS = T // GROUP         # MLP rows per core = 512
QC = 512                  # q-chunk width
NQC = T // QC             # 4
EPS = 1e-5
HID = 4 * N_EMBD          # 3072
NHC = HID // 128          # 24 hidden chunks
NDC = N_EMBD // 128       # 6 d chunks
NRT = T // 128            # 16 row tiles

GROUPS = [[0, 1, 2, 3], [4, 5, 6, 7]]

# HW-validated configuration (bisected on hardware 2026-08-08):
# - rank-1 PE matmul broadcast for the softmax denominator: OK
# - mixed-dtype tensor_tensor (psum f32 x bf16, f32 + bf16): OK
# - partition-shifted DVE writes (packed ctx): CRASHES real HW - keep off
USE_MM_BCAST = True
USE_PACKED_CTX = False
USE_MIXED_TT = True


def _split_multi_waits(nc, max_waits=1):
    """Split instructions with >max_waits sem-waits into preceding same-engine
    NoOps (this walrus build rejects multi-wait instructions)."""
    n = 0
    for f in nc.m.functions:
        for bb in f.blocks:
            out = []
            for ins in bb.instructions:
                si = ins.sync_info
                waits = list(si.on_wait) if si is not None else []
                if len(waits) > max_waits:
                    extra, keep = waits[:-max_waits], waits[-max_waits:]
                    for ci in range(0, len(extra), max_waits):
                        nop = mybir.InstNoOp(
                            name=f"{ins.name}-wsplit{ci}",
                            engine=ins.engine,
                            sync_info=mybir.SyncInfo(
                                on_wait=extra[ci:ci + max_waits], on_update=[]),
                            bass_nofuse=True,
                        )
                        out.append(nop)
                        n += 1
                    ins.sync_info = mybir.SyncInfo(
                        on_wait=keep, on_update=list(si.on_update))
                out.append(ins)
            bb.instructions = out
    return n


def build_program(repeat=1):
    nc = bass.Bass(num_devices=8)

    # ---------------- DRAM I/O ----------------
    x_d = nc.declare_dram_parameter("x", [T, N_EMBD], BF16, isOutput=False)
    wqkv_d = nc.declare_dram_parameter("wqkv", [128, NDC, 3 * 192], BF16, isOutput=False)
    bqkv_d = nc.declare_dram_parameter("bqkv", [128, 6], F32, isOutput=False)
    bv_d = nc.declare_dram_parameter("bv", [HPC * HD], F32, isOutput=False)
    pw_d = nc.declare_dram_parameter("pw", [HPC * HD, N_EMBD], BF16, isOutput=False)
    fw_d = nc.declare_dram_parameter("fw", [NHC, 128, NDC, 128], BF16, isOutput=False)
    fbt_d = nc.declare_dram_parameter("fbt", [128, NHC], F32, isOutput=False)
    f2w_d = nc.declare_dram_parameter("f2w", [HID, N_EMBD], BF16, isOutput=False)
    f2b_d = nc.declare_dram_parameter("f2b", [N_EMBD], F32, isOutput=False)
    mask_d = nc.declare_dram_parameter("mask", [128, 128], BF16, isOutput=False)
    xown_d = nc.declare_dram_parameter("xown", [ROWS, N_EMBD], F32, isOutput=False)
    out_d = nc.declare_dram_parameter("out", [ROWS, N_EMBD], F32, isOutput=True)

    dn_dram = nc.dram_tensor("dn_dram", [HPC, T], F32)
    proj_slab = [nc.dram_tensor(f"proj_slab{i}", [QC, N_EMBD], BF16)
                 for i in range(NQC)]
    rs_slab = [nc.dram_tensor(f"rs_slab{i}", [128, N_EMBD], BF16)
               for i in range(NQC)]

    def bcast_ap(dram_ap, p):
        return bass.AP(tensor=dram_ap.tensor, offset=dram_ap.offset,
                       ap=[[0, p]] + [list(d) for d in dram_ap.ap])

    with tile.TileContext(nc) as tc, ExitStack() as ctx:
        singles = ctx.enter_context(tc.tile_pool(name="singles", bufs=1))

        ident_raw = singles.tile([128, 128], F32, tag="ident_raw")
        make_identity(nc, ident_raw[:])
        ident = singles.tile([128, 128], BF16, tag="ident")
        nc.vector.tensor_copy(out=ident[:], in_=ident_raw[:])
        eps_t = singles.tile([128, 1], F32, tag="eps")
        nc.vector.memset(eps_t[:], EPS)
        ones3 = singles.tile([128, HPC], BF16, tag="ones3")
        nc.vector.memset(ones3[:], 1.0)
        ones64 = singles.tile([128, 64], BF16, tag="ones64")
        nc.vector.memset(ones64[:], 1.0)
        mask_sb = singles.tile([128, 128], BF16, tag="mask")
        nc.gpsimd.dma_start(mask_sb[:], mask_d[:, :])
        bqkv_sb = singles.tile([128, 6], F32, tag="bqkv")
        nc.gpsimd.dma_start(bqkv_sb[:], bqkv_d[:, :])
        bv_b = singles.tile([128, HPC * HD], F32, tag="bv")
        nc.gpsimd.dma_start(bv_b[:], bcast_ap(bv_d[:], 128))
        fbt_sb = singles.tile([128, NHC], F32, tag="fbt")
        nc.gpsimd.dma_start(fbt_sb[:], fbt_d[:, :])
        f2b_b = singles.tile([128, N_EMBD], F32, tag="f2b")
        nc.gpsimd.dma_start(f2b_b[:], bcast_ap(f2b_d[:], 128))
        wqkv_sb = singles.tile([128, NDC, 3 * 192], BF16, tag="wqkv")
        nc.sync.dma_start(wqkv_sb[:], wqkv_d[:, :, :])
        # pwH[1] lives at partitions 64:128 so proj's lhsT/rhs bases match
        # the packed ctx layout (ctxA: h0 low, h1 high; ctxB: h2 low).
        if USE_PACKED_CTX:
            pw01 = singles.tile([128, N_EMBD], BF16, tag="pw01")
            nc.gpsimd.dma_start(pw01[0:64, :], pw_d[0:HD, :])
            nc.gpsimd.dma_start(pw01[64:128, :], pw_d[HD:2 * HD, :])
            pw2 = singles.tile([128, N_EMBD], BF16, tag="pw2")
            nc.gpsimd.dma_start(pw2[0:64, :], pw_d[2 * HD:3 * HD, :])
            pwH = [pw01, pw01, pw2]
        else:
            pwH = []
            for h in range(HPC):
                t = singles.tile([128, N_EMBD], BF16, tag=f"pw{h}",
                                 name=f"pw{h}")
                nc.gpsimd.dma_start(t[0:64, :], pw_d[h * HD:(h + 1) * HD, :])
                pwH.append(t)
        # full fc1 weight prefetch [128, 24, 6, 128] bf16 (36KB/partition);
        # issued from the ACT queue so it can't block SP-issued x loads
        fw_sb = singles.tile([128, NHC, NDC, 128], BF16, tag="fw")
        for half in range(2):
            nc.scalar.dma_start(
                fw_sb[:, half * 12:(half + 1) * 12, :, :],
                fw_d[half * 12:(half + 1) * 12, :, :, :].rearrange(
                    "a p b m -> p a b m"))
        # f2w fully resident too (36KB/partition): [128, 24, 768] bf16,
        # hid-chunk hc at [:, hc, :]
        f2w_sb = singles.tile([128, NHC, N_EMBD], BF16, tag="f2w")
        for half in range(2):
            nc.scalar.dma_start(
                f2w_sb[:, half * 12:(half + 1) * 12, :],
                f2w_d[half * 12 * 128:(half + 1) * 12 * 128, :].rearrange(
                    "(a p) m -> p a m", p=128))

        for _rep in range(repeat):
          with ExitStack() as rep_ctx:
            # ---- persistent per-rep pools ----
            p_qk = rep_ctx.enter_context(tc.tile_pool(name="p_qk", bufs=1))
            p_mlp = rep_ctx.enter_context(tc.tile_pool(name="p_mlp", bufs=1))
            # qA/kA: heads 0,1 on partition halves; qB/kB: head 2 at [0:64].
            # ctxA: h0/h1 on halves; ctxB: h2 at [0:64].
            qAg = [p_qk.tile([128, QC], BF16, tag=f"qA{g}", name=f"qA{g}")
                   for g in range(NQC)]
            qBg = [p_qk.tile([128, QC], BF16, tag=f"qB{g}", name=f"qB{g}")
                   for g in range(NQC)]
            kAg = [p_qk.tile([128, QC], BF16, tag=f"kA{g}", name=f"kA{g}")
                   for g in range(NQC)]
            kBg = [p_qk.tile([128, QC], BF16, tag=f"kB{g}", name=f"kB{g}")
                   for g in range(NQC)]
            vaug = [p_qk.tile([128, HPC, HD + 1], BF16, tag=f"vaug{s}",
                              name=f"vaug{s}") for s in range(NRT)]
            if USE_PACKED_CTX:
                ctxA = p_qk.tile([128, T], BF16, tag="ctxA", name="ctxA")
                ctxB = p_qk.tile([128, T], BF16, tag="ctxB", name="ctxB")
                ctx_tiles = [(ctxA, 0), (ctxA, 64), (ctxB, 0)]
            else:
                ctx_tiles = [
                    (p_qk.tile([128, T], BF16, tag=f"ctx{h}", name=f"ctx{h}"), 0)
                    for h in range(HPC)]

            def ctx_sl(h):
                return ctx_tiles[h]
            x2 = [p_mlp.tile([128, N_EMBD], F32, tag=f"x2_{rt}", name=f"x2_{rt}")
                  for rt in range(NQC)]
            x2nT = p_mlp.tile([128, NDC, ROWS], BF16, tag="x2nT", name="x2nT")
            hT = p_mlp.tile([128, NHC, ROWS], BF16, tag="hT", name="hT")

            with ExitStack() as a_ctx:
                # ---- attention-phase PSUM: exactly 8 banks via tag sharing
                # "sc" (scores+qkv) 2, "av" 2, "ptf" (transposes+fc1) 2,
                # "pp" (proj+vnat) 2.  PSUM slots are bank-granular.
                ps512 = a_ctx.enter_context(
                    tc.tile_pool(name="ps512", bufs=2, space="PSUM"))
                psA = a_ctx.enter_context(
                    tc.tile_pool(name="psA", bufs=2, space="PSUM"))
                psT = ps512
                psP = ps512
                psF1 = ps512
                xpool = a_ctx.enter_context(tc.tile_pool(name="xpool", bufs=2))
                tpool = a_ctx.enter_context(tc.tile_pool(name="tpool", bufs=2))
                spool = a_ctx.enter_context(tc.tile_pool(name="spool", bufs=4))
                epool = a_ctx.enter_context(tc.tile_pool(name="epool", bufs=4))
                rpool = a_ctx.enter_context(tc.tile_pool(name="rpool", bufs=2))
                dpool = a_ctx.enter_context(tc.tile_pool(name="dpool", bufs=2))
                ppool = a_ctx.enter_context(tc.tile_pool(name="ppool", bufs=1))
                mpool = a_ctx.enter_context(tc.tile_pool(name="mpool", bufs=1))

                def layer_norm_tile(xt, out_bf, spool_):
                    """LN over free dim 768 of [128,768] xt -> out_bf (bf16)."""
                    stats = spool_.tile([128, 3, 6], F32, tag="bn_st")
                    xgr = xt.rearrange("p (g c) -> p g c", g=3)
                    for g in range(3):
                        nc.vector.bn_stats(out=stats[:, g, :], in_=xgr[:, g, :])
                    mv = spool_.tile([128, 2], F32, tag="bn_mv")
                    nc.vector.bn_aggr(out=mv[:], in_=stats[:])
                    sd = spool_.tile([128, 1], F32, tag="bn_sd")
                    nc.scalar.activation(sd[:], mv[:, 1:2], AF.Sqrt,
                                         bias=eps_t[:])
                    nc.vector.reciprocal(sd[:], sd[:])
                    nc.vector.tensor_scalar(
                        out=out_bf, in0=xt, scalar1=mv[:, 0:1],
                        scalar2=sd[:], op0=OP.subtract, op1=OP.mult)

                def transpose6(src_bf, dst, dst_col):
                    """Transpose [128,768] bf16 src into dst[:, 0:6, dst_col:+128].
                    All 6 bf16 transposes fit one PSUM bank; single copy out."""
                    pt = psA.tile([128, NDC * 128], BF16, tag="av")
                    for j in range(NDC):
                        nc.tensor.transpose(
                            pt[:, j * 128:(j + 1) * 128],
                            src_bf[:, j * 128:(j + 1) * 128], ident[:])
                    nc.vector.tensor_copy(
                        out=dst[:, 0:NDC, dst_col:dst_col + 128],
                        in_=pt[:].rearrange("p (b m) -> p b m", b=NDC))

                def qkv_block(rg):
                    xnT = tpool.tile([128, NDC, QC], BF16, tag="xnT",
                                     name=f"xnT_{rg}")
                    for rt4 in range(4):
                        xg = xpool.tile([128, N_EMBD], BF16, tag="x_in")
                        nc.sync.dma_start(
                            xg[:], x_d[rg * 512 + rt4 * 128:
                                       rg * 512 + (rt4 + 1) * 128, :])
                        xn = xpool.tile([128, N_EMBD], BF16, tag="x_n")
                        layer_norm_tile(xg[:], xn[:], spool)
                        transpose6(xn[:], xnT, rt4 * 128)
                    # q,k in transposed [hd, rows] layout:
                    # (dst, partition offset, mdim, weight col offset, bias col)
                    plan = [
                        (qAg[rg], 0, 128, 0, 0),      # q heads 0,1
                        (qBg[rg], 0, 64, 128, 1),     # q head 2
                        (kAg[rg], 0, 128, 192, 2),    # k heads 0,1
                        (kBg[rg], 0, 64, 320, 3),     # k head 2
                    ]
                    for i, (dst, po, mdim, moff, bcol) in enumerate(plan):
                        ps = psA.tile([128, QC], F32, tag="av")
                        for dc in range(NDC):
                            nc.tensor.matmul(
                                ps[po:po + mdim, :],
                                wqkv_sb[:, dc, moff:moff + mdim],
                                xnT[:, dc, :],
                                start=(dc == 0), stop=(dc == NDC - 1))
                        bias = bqkv_sb[po:po + mdim, bcol:bcol + 1]
                        if i == 0:
                            nc.scalar.activation(
                                dst[po:po + mdim, :], ps[po:po + mdim, :],
                                AF.Identity, bias=bias)
                        else:
                            nc.vector.tensor_scalar_add(
                                out=dst[po:po + mdim, :],
                                in0=ps[po:po + mdim, :], scalar1=bias)
                    # v in natural [rows, (h, hd)] layout, bias fused in copy
                    for rt4 in range(4):
                        s = rg * 4 + rt4
                        ps = psP.tile([128, HPC * HD], F32, tag="pp")
                        for dc in range(NDC):
                            nc.tensor.matmul(
                                ps[:], xnT[:, dc, rt4 * 128:(rt4 + 1) * 128],
                                wqkv_sb[:, dc, 384:576],
                                start=(dc == 0), stop=(dc == NDC - 1))
                        nc.vector.tensor_tensor(
                            out=vaug[s][:, :, 0:HD],
                            in0=ps[:].rearrange("p (h d) -> p h d", h=HPC),
                            in1=bv_b[:].rearrange("p (h d) -> p h d", h=HPC),
                            op=OP.add)
                        nc.vector.tensor_copy(out=vaug[s][:, :, HD], in_=ones3[:])

                def ksl(h, s):
                    t = kAg[s // 4] if h < 2 else kBg[s // 4]
                    po = 64 if h == 1 else 0
                    return t[po:po + 64, (s % 4) * 128:(s % 4 + 1) * 128]

                def qsl(h, qc, lo=0):
                    t = qAg[qc] if h < 2 else qBg[qc]
                    po = 64 if h == 1 else 0
                    return t[po:po + 64, lo:QC]

                def attn_head(h, qc, stag=2):
                    ns = 4 * (qc + 1)
                    pav = psA.tile([128, QC], F32, tag="av")
                    ets = {}
                    offs = {}

                    def av(s):
                        lo = offs[s]
                        nc.tensor.matmul(
                            pav[0:HD + 1, lo:QC], vaug[s][:, h, :],
                            ets.pop(s)[:, lo:QC],
                            start=(s == 0), stop=(s == ns - 1))

                    for s in range(ns):
                        voff = s - (ns - 4)      # >=0 on diagonal chunks
                        lo = max(voff, 0) * 128
                        offs[s] = lo
                        ps = ps512.tile([128, QC], F32, tag="sc")
                        nc.tensor.matmul(
                            ps[:, lo:QC], ksl(h, s), qsl(h, qc, lo),
                            start=True, stop=True)
                        et = epool.tile([128, QC], BF16, tag="exp")
                        nc.scalar.activation(et[:, lo:QC], ps[:, lo:QC], AF.Exp)
                        if voff >= 0:
                            nc.vector.tensor_tensor(
                                out=et[:, lo:lo + 128], in0=et[:, lo:lo + 128],
                                in1=mask_sb[:], op=OP.mult)
                        ets[s] = et
                        if s >= stag:
                            av(s - stag)
                    for s in range(max(ns - stag, 0), ns):
                        av(s)
                    rb = rpool.tile([128, QC], BF16, tag="rbq")
                    if USE_MM_BCAST:
                        # denominator broadcast via a rank-1 PE matmul
                        # (ones[1,64]^T x den[1,512] -> psum[64,512]); no DMA.
                        dstg = dpool.tile([128, QC], BF16, tag="dstage")
                        nc.vector.tensor_copy(
                            out=dstg[HD:HD + 1, :], in_=pav[HD:HD + 1, :])
                        pbc = psT.tile([128, QC], F32, tag="ptf")
                        nc.tensor.matmul(
                            pbc[0:64, :], ones64[HD:HD + 1, :],
                            dstg[HD:HD + 1, :], start=True, stop=True)
                        with nc.allow_low_precision(reason="bf16 softmax recip"):
                            nc.vector.reciprocal(out=rb[0:64, :],
                                                 in_=pbc[0:64, :])
                    else:
                        dstg = dpool.tile([128, QC], F32, tag="dstage")
                        nc.vector.tensor_copy(
                            out=dstg[HD:HD + 1, :], in_=pav[HD:HD + 1, :])
                        nc.sync.dma_start(
                            dn_dram[h, qc * QC:(qc + 1) * QC],
                            dstg[HD:HD + 1, :])
                        rbf = dpool.tile([128, QC], F32, tag="rbf")
                        nc.sync.dma_start(
                            rbf[0:64, :],
                            bcast_ap(dn_dram[h, qc * QC:(qc + 1) * QC], 64))
                        with nc.allow_low_precision(reason="bf16 softmax recip"):
                            nc.vector.reciprocal(out=rb[0:64, :],
                                                 in_=rbf[0:64, :])
                    ct, po = ctx_sl(h)
                    if USE_MIXED_TT:
                        # fused normalize: ctx = pav(psum f32) * rb(bf16)
                        nc.vector.tensor_tensor(
                            out=ct[po:po + HD, qc * QC:(qc + 1) * QC],
                            in0=pav[0:HD, :], in1=rb[0:64, :], op=OP.mult)
                    else:
                        nc.vector.tensor_copy(
                            out=ct[po:po + HD, qc * QC:(qc + 1) * QC],
                            in_=pav[0:HD, :])
                        nc.vector.tensor_tensor(
                            out=ct[po:po + HD, qc * QC:(qc + 1) * QC],
                            in0=ct[po:po + HD, qc * QC:(qc + 1) * QC],
                            in1=rb[0:64, :], op=OP.mult)

                def proj_block(qc):
                    pp = ppool.tile([128, 4, N_EMBD], BF16, tag="pp")
                    for rc in range(4):
                        col = qc * QC + rc * 128
                        for ng in range(2):
                            ps = psP.tile([128, 384], F32, tag="pp")
                            for h in range(HPC):
                                ct, po = ctx_sl(h)
                                nc.tensor.matmul(
                                    ps[:], ct[po:po + 64, col:col + 128],
                                    pwH[h][po:po + 64, ng * 384:(ng + 1) * 384],
                                    start=(h == 0), stop=(h == HPC - 1))
                            if ng == 0:
                                nc.scalar.copy(out=pp[:, rc, 0:384], in_=ps[:])
                            else:
                                nc.vector.tensor_copy(
                                    out=pp[:, rc, 384:768], in_=ps[:])
                    nc.sync.dma_start(
                        proj_slab[qc][:, :].rearrange("(a p) m -> p a m", p=128),
                        pp[:])
                    nc.gpsimd.collective_compute(
                        "ReduceScatter", OP.add, replica_groups=GROUPS,
                        ins=[proj_slab[qc].ap().opt()],
                        outs=[rs_slab[qc].ap().opt()])

                def early_mlp(rt):
                    """Residual + LN2 + transpose for own row tile rt
                    (depends on rs_slab[rt])."""
                    xo = mpool.tile([128, N_EMBD], F32, tag="xo")
                    nc.sync.dma_start(
                        xo[:], xown_d[rt * 128:(rt + 1) * 128, :])
                    rsb = mpool.tile([128, N_EMBD], BF16, tag="rsb")
                    nc.gpsimd.dma_start(rsb[:], rs_slab[rt][:, :])
                    if USE_MIXED_TT:
                        nc.vector.tensor_tensor(
                            out=x2[rt][:], in0=xo[:], in1=rsb[:], op=OP.add)
                    else:
                        rsf = mpool.tile([128, N_EMBD], F32, tag="rsf")
                        nc.vector.tensor_copy(out=rsf[:], in_=rsb[:])
                        nc.vector.tensor_tensor(
                            out=x2[rt][:], in0=xo[:], in1=rsf[:], op=OP.add)
                    x2n = mpool.tile([128, N_EMBD], BF16, tag="x2n")
                    layer_norm_tile(x2[rt][:], x2n[:], spool)
                    transpose6(x2n[:], x2nT, rt * 128)

                def fc1_tile(rt):
                    """fc1 + GELU for one 128-row tile (bf16 matmuls have no
                    free-dim penalty, so per-tile fc1 is gated only by RS rt)."""
                    cols = slice(rt * 128, (rt + 1) * 128)
                    for hc in range(NHC):
                        ps = psF1.tile([128, 128], F32, tag="ptf")
                        for dc in range(NDC):
                            nc.tensor.matmul(
                                ps[:], fw_sb[:, hc, dc, :],
                                x2nT[:, dc, cols],
                                start=(dc == 0), stop=(dc == NDC - 1))
                        nc.scalar.activation(
                            hT[:, hc, cols], ps[:], AF.Gelu_apprx_tanh,
                            bias=fbt_sb[:, hc:hc + 1])

                def fc1_half01(hcs):
                    """fc1 + GELU for row tiles 0,1 (cols 0:256, needs RS0+RS1)
                    over an hc subrange — wider gelus halve the ACT overhead."""
                    for hc in hcs:
                        ps = psF1.tile([128, 256], F32, tag="ptf")
                        for dc in range(NDC):
                            nc.tensor.matmul(
                                ps[:], fw_sb[:, hc, dc, :],
                                x2nT[:, dc, 0:256],
                                start=(dc == 0), stop=(dc == NDC - 1))
                        nc.scalar.activation(
                            hT[:, hc, 0:256], ps[:], AF.Gelu_apprx_tanh,
                            bias=fbt_sb[:, hc:hc + 1])

                # ---------- the interleaved schedule ----------
                # The attention->proj->ReduceScatter chain is kept hot; MLP
                # filler work (residual/LN2/fc1 per landed row tile) is issued
                # right after each RS so it fills engine gaps without ever
                # blocking the chain.
                qkv_block(0)
                for h in range(HPC):
                    attn_head(h, 0)
                proj_block(0)

                qkv_block(1)
                for h in range(HPC):
                    attn_head(h, 1)
                proj_block(1)

                qkv_block(2)
                early_mlp(0)
                for h in range(HPC):
                    attn_head(h, 2)
                proj_block(2)
                early_mlp(1)
                fc1_half01(range(0, 12))

                qkv_block(3)
                for h in range(HPC):
                    attn_head(h, 3)
                proj_block(3)
                fc1_half01(range(12, NHC))
                early_mlp(2)
                fc1_tile(2)

            # ---------- fc2 + tail (early_mlp(3), fc1(t3)) + epilogue ----
            # fc2(rt) depends only on fc1_tile(rt): rt 0-2 cover the RS3 wait.
            with ExitStack() as g_ctx:
                psF2 = g_ctx.enter_context(
                    tc.tile_pool(name="psF2", bufs=1, space="PSUM"))
                opool = g_ctx.enter_context(tc.tile_pool(name="opool", bufs=4))
                spool2 = g_ctx.enter_context(tc.tile_pool(name="spool2", bufs=4))
                mpool2 = g_ctx.enter_context(tc.tile_pool(name="mpool2", bufs=2))

                def mlp3_pre(rt):
                    """Residual + LN2 for tile rt — DVE/ACT only, so it fills
                    the RS3 wait without stalling the PE queue."""
                    xo = mpool2.tile([128, N_EMBD], F32, tag="xo")
                    nc.sync.dma_start(
                        xo[:], xown_d[rt * 128:(rt + 1) * 128, :])
                    rsb = mpool2.tile([128, N_EMBD], BF16, tag="rsb")
                    nc.gpsimd.dma_start(rsb[:], rs_slab[rt][:, :])
                    if USE_MIXED_TT:
                        nc.vector.tensor_tensor(
                            out=x2[rt][:], in0=xo[:], in1=rsb[:], op=OP.add)
                    else:
                        rsf = mpool2.tile([128, N_EMBD], F32, tag="rsf")
                        nc.vector.tensor_copy(out=rsf[:], in_=rsb[:])
                        nc.vector.tensor_tensor(
                            out=x2[rt][:], in0=xo[:], in1=rsf[:], op=OP.add)
                    x2n = mpool2.tile([128, N_EMBD], BF16, tag="x2n")
                    stats = spool2.tile([128, 3, 6], F32, tag="bn_st")
                    xgr = x2[rt][:].rearrange("p (g c) -> p g c", g=3)
                    for g in range(3):
                        nc.vector.bn_stats(out=stats[:, g, :], in_=xgr[:, g, :])
                    mv = spool2.tile([128, 2], F32, tag="bn_mv")
                    nc.vector.bn_aggr(out=mv[:], in_=stats[:])
                    sd = spool2.tile([128, 1], F32, tag="bn_sd")
                    nc.scalar.activation(sd[:], mv[:, 1:2], AF.Sqrt,
                                         bias=eps_t[:])
                    nc.vector.reciprocal(sd[:], sd[:])
                    nc.vector.tensor_scalar(
                        out=x2n[:], in0=x2[rt][:], scalar1=mv[:, 0:1],
                        scalar2=sd[:], op0=OP.subtract, op1=OP.mult)
                    return x2n

                def mlp3_post(rt, x2n):
                    pt = psF2.tile([128, NDC * 128], BF16, tag="ptf2", bufs=2)
                    for j in range(NDC):
                        nc.tensor.transpose(
                            pt[:, j * 128:(j + 1) * 128],
                            x2n[:, j * 128:(j + 1) * 128], ident[:])
                    nc.vector.tensor_copy(
                        out=x2nT[:, 0:NDC, rt * 128:(rt + 1) * 128],
                        in_=pt[:].rearrange("p (b m) -> p b m", b=NDC))

                def fc1_tile2(rt):
                    cols = slice(rt * 128, (rt + 1) * 128)
                    for hc in range(NHC):
                        ps = psF2.tile([128, 128], F32, tag="ptf2", bufs=2)
                        for dc in range(NDC):
                            nc.tensor.matmul(
                                ps[:], fw_sb[:, hc, dc, :],
                                x2nT[:, dc, cols],
                                start=(dc == 0), stop=(dc == NDC - 1))
                        nc.scalar.activation(
                            hT[:, hc, cols], ps[:], AF.Gelu_apprx_tanh,
                            bias=fbt_sb[:, hc:hc + 1])

                def fc2_tile(rt):
                    """fc2 + epilogue for one row tile; needs only fc1(rt)."""
                    pss = {}
                    for ng in range(2):
                        pss[ng] = psF2.tile(
                            [128, 384], F32, tag="fc2", bufs=4,
                            name=f"fc2ps_{ng}_{rt}")
                    for hc in range(NHC):
                        for ng in range(2):
                            nc.tensor.matmul(
                                pss[ng][:],
                                hT[:, hc, rt * 128:(rt + 1) * 128],
                                f2w_sb[:, hc, ng * 384:(ng + 1) * 384],
                                start=(hc == 0), stop=(hc == NHC - 1))
                    ot = opool.tile([128, N_EMBD], F32, tag="ot")
                    for ng in range(2):
                        nc.vector.tensor_tensor(
                            out=ot[:, ng * 384:(ng + 1) * 384],
                            in0=pss[ng][:],
                            in1=f2b_b[:, ng * 384:(ng + 1) * 384],
                            op=OP.add)
                    nc.gpsimd.tensor_tensor(
                        out=ot[:], in0=ot[:], in1=x2[rt][:], op=OP.add)
                    nc.sync.dma_start(
                        out_d[rt * 128:(rt + 1) * 128, :], ot[:])

                x2n3 = mlp3_pre(3)   # DVE work fills the RS3 wait
                fc2_tile(0)          # rt0-2 PE work covers the RS3 window
                fc2_tile(1)
                fc2_tile(2)
                mlp3_post(3, x2n3)
                fc1_tile2(3)
                fc2_tile(3)

    _split_multi_waits(nc, max_waits=1)
    return nc


def _host_prep(inputs):
    """Fold LN affines into weights; build per-core input maps (bf16)."""
    x = np.ascontiguousarray(np.asarray(inputs["x"], dtype=np.float32))
    aw = np.asarray(inputs["attn_w"], np.float32) * np.asarray(inputs["ln1_w"], np.float32)[:, None]
    ab = np.asarray(inputs["attn_b"], np.float32) + np.asarray(inputs["ln1_b"], np.float32) @ np.asarray(inputs["attn_w"], np.float32)
    aw = aw.copy()
    ab = ab.copy()
    aw[:, :N_EMBD] *= 0.125
    ab[:N_EMBD] *= 0.125
    fw = np.asarray(inputs["fc_w"], np.float32) * np.asarray(inputs["ln2_w"], np.float32)[:, None]
    fb = np.asarray(inputs["fc_b"], np.float32) + np.asarray(inputs["ln2_b"], np.float32) @ np.asarray(inputs["fc_w"], np.float32)
    f2w = np.asarray(inputs["fc2_w"], np.float32)
    f2b = np.asarray(inputs["fc2_b"], np.float32)
    pw_full = np.asarray(inputs["proj_w"], np.float32)
    pb = np.asarray(inputs["proj_b"], np.float32)

    # multiplicative causal mask for the mixed 128x128 diagonal block
    p = np.arange(128)
    f = np.arange(128)
    mask = (f[None, :] >= p[:, None]).astype(NPBF16)

    # fw device layout [NHC, 128, NDC, 128]
    fw_dev = np.ascontiguousarray(
        fw.reshape(NDC, 128, NHC, 128).transpose(2, 1, 0, 3).astype(NPBF16))
    fbt = np.ascontiguousarray(fb.reshape(NHC, 128).T)  # [128, NHC]

    xbf = x.astype(NPBF16)
    in_maps = []
    for core in range(8):
        b = core // GROUP
        r = core % GROUP
        hsl = slice(r * HPC * HD, (r + 1) * HPC * HD)
        wq = aw[:, 0:N_EMBD][:, hsl]
        wk = aw[:, N_EMBD:2 * N_EMBD][:, hsl]
        wv = aw[:, 2 * N_EMBD:][:, hsl]
        wqkv = np.concatenate([wq, wk, wv], axis=1)  # [768, 576]
        wqkv_dev = np.ascontiguousarray(
            wqkv.reshape(NDC, 128, 576).transpose(1, 0, 2).astype(NPBF16))
        bq = ab[0:N_EMBD][hsl]
        bk = ab[N_EMBD:2 * N_EMBD][hsl]
        bv = np.ascontiguousarray(ab[2 * N_EMBD:][hsl])
        bqkv = np.zeros((128, 6), np.float32)
        for sec, bb_ in enumerate([bq, bk]):
            bqkv[:, sec * 2] = bb_[0:128]
            bqkv[:64, sec * 2 + 1] = bb_[128:192]
        # own rows: strided {qc*512 + r*128 + j}; proj bias pre-added
        own = np.concatenate(
            [np.arange(qc * 512 + r * 128, qc * 512 + r * 128 + 128)
             for qc in range(NQC)])
        xown = np.ascontiguousarray(x[b][own] + pb[None, :])
        in_maps.append({
            "x": xbf[b],
            "xown": xown,
            "wqkv": wqkv_dev,
            "bqkv": bqkv,
            "bv": bv,
            "pw": np.ascontiguousarray(pw_full[hsl, :].astype(NPBF16)),
            "fw": fw_dev,
            "fbt": fbt,
            "f2w": np.ascontiguousarray(f2w.astype(NPBF16)),
            "f2b": f2b,
            "mask": mask,
        })
    return in_maps


@lru_cache(maxsize=1)
def _get_program():
    return build_program()


def kernel(**inputs):
    in_maps = _host_prep(inputs)
    nc = _get_program()
    res = run_bass_kernel_spmd(nc, in_maps, list(range(8)))
    out = np.zeros((B, T, N_EMBD), np.float32)
    for core in range(8):
        b, r = core // GROUP, core % GROUP
        for qc in range(NQC):
            out[b, qc * 512 + r * 128:qc * 512 + (r + 1) * 128] = \
                res.results[core]["out"][qc * 128:(qc + 1) * 128]
    return out
